# revision 14
# baseline (speedup 1.0000x reference)
"""Trainium2 Bass kernel for nn_Classifier (capsule-style conv + routing).

Math (validated against the jax reference):
  W = conv_w[:,0,:]                                   # [16, 640]
  y[b,i,o]   = relu(sum_t x[b,i,t] W[t,o] + conv_b[o])          (conv as matmul, K=16)
  U[b,k,i,d] = y[b,i,k*64+d]
  Usum[b,k,d]= sum_i U[b,k,i,d]
  logits     = (U . Usum)/4            -> stable softmax over i  -> C
  Cb         = C + B_bias[k,i]
  S[b,k,:]   = sum_i Cb[b,k,i] U[b,k,i,:]
  out[b,k]   = n2/(n2+1),  n2 = |S|^2     (sqrt factor n/(n+eps) ~ 1, err < 1e-5)

Sharding: data-parallel over batch, 8 batches per core, 8 cores (SPMD).

Per-core design (b = 4g+j, g in {0,1} bgroups, j in 0..3):
  - constants (identity / gmask / smask / ones) precomputed on host, DMA'd in
  - x loaded naturally, PE-transposed (17-col chunks incl. a ones column for
    the bias fold) into xT4[g] [128,512]: rows 32j+0=1.0, 32j+1+t = x[b,:,t]
  - conv computed in BOTH orientations on PE (fp32r, 4x row-tiled via
    tile_position): yr_oi[b] [o-chunk(128) x i(512)] and yr_io[b]
    [i-chunk(128) x o(640)]
  - PSUM->SBUF relu evictions split across DVE and ACT (the hard bottleneck:
    PSUM reads are 1 elem/lane/cycle on each); usum via accum_out on the oi
    evictions; gmat/Cb/squash-accum offloaded to GpSimd (SBUF-only engine)
  - logits = gmat^T @ yr_oi (col-tiled 4x, accumulated over 5 o-chunks),
    softmax via reduce_max(negate) + exp(bias=-max, accum=Z), Cb=C/Z+B
  - Cb transposed on PE, S = sum_q CbT_q^T @ yr_io_q (col-tiled 4x)
  - PSUM: one 4-bank ring shared by oi/logits/CbT/S tiles + one 2x2-bank ring
    for io tiles = exactly 8 banks, no pool-scope barriers mid-kernel
"""

import numpy as np

import concourse.bass as bass
import concourse.mybir as mybir
import concourse.tile as tile
from concourse import bacc
from concourse.bass_utils import run_bass_kernel_spmd

F32 = mybir.dt.float32
F32R = mybir.dt.float32r

B_FULL = 64
N = 512          # num timecaps (routing dim m/i)
DT = 16          # dim timecaps (conv contraction)
K = 10           # classes
D = 64           # dim classes
NO = K * D       # 640 conv output channels
NCORES = 8
BPC = B_FULL // NCORES   # 8 batches per core

# consts layout (columns of the [128, CW] host-precomputed block)
C_IDENT = 0      # [128] identity for PE transpose
C_ONES = 128     # [4] ones columns for the xn bias fold
C_GMASK = 132    # [50] gmask: 0.25 at class(128c+p)==k
C_SMASK = 182    # [640] rows 32j+k: 1.0 at cols [64k,64k+64)
CW = 822

# eviction engine split (True -> DVE, False -> ACT), tuned for balance
OI_DVE_PAT = (1, 0, 1, 0, 1, 0, 1, 1)   # 5/8 of oi tiles on DVE
IO_DVE_PAT = (1, 0, 0, 1, 0, 0, 0, 0)   # 2/8 of io tiles on DVE


def _consts_np() -> np.ndarray:
    c = np.zeros((128, CW), np.float32)
    c[:, C_IDENT:C_IDENT + 128] = np.eye(128, dtype=np.float32)
    c[:, C_ONES:C_ONES + 4] = 1.0
    for cc in range(5):
        for p in range(128):
            k = (128 * cc + p) // 64
            c[p, C_GMASK + 10 * cc + k] = 0.25
    for j in range(4):
        for k in range(K):
            c[32 * j + k, C_SMASK + 64 * k:C_SMASK + 64 * (k + 1)] = 1.0
    return c


def _build_program():
    nc = bacc.Bacc("TRN2", target_bir_lowering=False)
    x_in = nc.declare_dram_parameter("x", [BPC, N, DT], F32, isOutput=False)
    w_in = nc.declare_dram_parameter("w", [DT, 1, NO], F32, isOutput=False)
    cb_in = nc.declare_dram_parameter("cb", [NO], F32, isOutput=False)
    bb_in = nc.declare_dram_parameter("bb", [K, 1, N], F32, isOutput=False)
    cst_in = nc.declare_dram_parameter("cst", [128, CW], F32, isOutput=False)
    out_d = nc.declare_dram_parameter("out", [BPC, K], F32, isOutput=True)

    AF = mybir.ActivationFunctionType
    OP = mybir.AluOpType

    with tile.TileContext(nc) as tc:
        with tc.tile_pool(name="sb", bufs=1) as sb:
            # ---- SBUF persistent tiles ----
            csb = sb.tile([128, CW], F32, name="csb", tag="csb")
            w4 = sb.tile([128, NO], F32R, name="w4", tag="w4")
            wstage = sb.tile([128, NO], F32, name="wstage", tag="wstage")
            bbias = sb.tile([128, N], F32, name="bbias", tag="bbias")
            xc_all = sb.tile([128, BPC * 64], F32, name="xc_all", tag="xc_all")
            xT4 = [sb.tile([128, N], F32R, name=f"xT4_{g}", tag=f"xT4_{g}")
                   for g in range(2)]
            yr_oi = [sb.tile([128, 5 * N], F32, name=f"yoi{b}", tag=f"yoi{b}")
                     for b in range(BPC)]
            yr_io = [sb.tile([128, 4 * NO], F32, name=f"yio{b}", tag=f"yio{b}")
                     for b in range(BPC)]
            usum = [sb.tile([128, 5], F32, name=f"us{b}", tag=f"us{b}")
                    for b in range(BPC)]
            gmat = [sb.tile([128, 5 * K], F32, name=f"gm{b}", tag=f"gm{b}")
                    for b in range(BPC)]
            exp_sb = [sb.tile([128, N], F32, name=f"ex{g}", tag=f"ex{g}") for g in range(2)]
            cb_sb = [sb.tile([128, N], F32, name=f"cb{g}", tag=f"cb{g}") for g in range(2)]
            negmax = [sb.tile([128, 1], F32, name=f"nm{g}", tag=f"nm{g}") for g in range(2)]
            zsum = [sb.tile([128, 1], F32, name=f"z{g}", tag=f"z{g}") for g in range(2)]
            rz = [sb.tile([128, 1], F32, name=f"rz{g}", tag=f"rz{g}") for g in range(2)]
            ebt = [sb.tile([128, 4 * 4 * K], F32, name=f"ebt{g}", tag=f"ebt{g}")
                   for g in range(2)]
            sm_s = [sb.tile([128, NO], F32, name=f"sm{g}", tag=f"sm{g}") for g in range(2)]
            sq_s = [sb.tile([128, NO], F32, name=f"sq{g}", tag=f"sq{g}") for g in range(2)]
            n2 = sb.tile([128, 2], F32, name="n2", tag="n2")
            t_c = sb.tile([128, 2], F32, name="t_c", tag="t_c")
            t_d = sb.tile([128, 2], F32, name="t_d", tag="t_d")
            t_o = sb.tile([128, 2], F32, name="t_o", tag="t_o")
            wrm = sb.tile([128, N], F32R, name="wrm", tag="wrm")

            ident = csb[:, C_IDENT:C_IDENT + 128]
            gmask = csb[:, C_GMASK:C_GMASK + 50]
            smask = csb[:, C_SMASK:C_SMASK + NO]

            # ---- input DMAs ----
            nc.gpsimd.memset(wrm[:].bitcast(F32), 0.125)
            for g in range(2):
                nc.gpsimd.memset(xT4[g][:].bitcast(F32), 1.0)
            nc.sync.dma_start(csb[:, 0:128], cst_in[:, 0:128])      # ident first
            nc.sync.dma_start(csb[:, 128:CW], cst_in[:, 128:CW])
            wflat = w_in.rearrange("t u o -> (t u) o")
            cbflat = cb_in.rearrange("(u o) -> u o", u=1)
            for j in range(4):
                nc.sync.dma_start(wstage[32 * j:32 * j + 16, :], wflat[:, :])
                nc.sync.dma_start(wstage[32 * j + 16:32 * j + 17, :], cbflat[:, :])
                nc.sync.dma_start(
                    bbias[32 * j:32 * j + K, :],
                    bb_in.rearrange("k u m -> k (u m)"),
                )
            for h in range(2):
                eng = nc.gpsimd if h == 0 else nc.sync
                eng.dma_start(
                    xc_all[:, 256 * h:256 * (h + 1)].rearrange(
                        "p (b w) -> p b w", b=4),
                    x_in[4 * h:4 * (h + 1)].rearrange(
                        "b (p w) t -> p b (w t)", p=128),
                )

            nc.vector.tensor_copy(w4[:], wstage[:])

            # ---- x transposes: xn [i-part, 17u-chunks] -> xT4[g] [32j+c, i] ----
            with tc.tile_pool(name="px", bufs=4, space="PSUM") as px:
                ps_w = px.tile([128, N], F32, name="ps_w", tag="warm", bufs=1)
                for _ in range(12):
                    nc.tensor.matmul(
                        ps_w[:], wrm[0:17, 0:128], wrm[0:17, :],
                        start=True, stop=True,
                    )
                for g in range(2):
                    for j in range(4):
                        b = 4 * g + j
                        ps_x = px.tile([16, N], F32, name="ps_x", tag="ps_x")
                        for il in range(4):
                            nc.tensor.transpose(
                                ps_x[:, 128 * il:128 * (il + 1)],
                                xc_all[:, 64 * b + 16 * il:64 * b + 16 * il + 16],
                                ident,
                            )
                        # ps_x cols 128*il+p  ->  xT4 col 4*p+il (i = 4p+il)
                        dst = xT4[g][32 * j:32 * j + 16, :].rearrange(
                            "t (p il) -> t il p", il=4)
                        srcv = ps_x[:].rearrange("t (il p) -> t il p", il=4)
                        if j % 2 == 0:
                            nc.vector.tensor_copy(dst, srcv)
                        else:
                            nc.scalar.copy(dst, srcv)

            # ---- main PSUM rings: 4x1-bank (oi/logits/CbT/S) + 2x2-bank (io) ----
            with tc.tile_pool(name="poi", bufs=4, space="PSUM") as poi, \
                 tc.tile_pool(name="pio", bufs=2, space="PSUM") as pio:

                ei_oi = [0]
                ei_io = [0]

                def conv_oi(g):
                    for c in range(5):
                        for j in range(4):
                            b = 4 * g + j
                            ps = poi.tile([128, N], F32, name="ps", tag="ps")
                            nc.tensor.matmul(
                                ps[:],
                                w4[32 * j:32 * j + 17,
                                   128 * c:128 * (c + 1)],
                                xT4[g][32 * j:32 * j + 17, :],
                                start=True, stop=True,
                                tile_position=(32 * j, 0),
                            )
                            dst = yr_oi[b][:, N * c:N * (c + 1)]
                            acc = usum[b][:, c:c + 1]
                            if OI_DVE_PAT[ei_oi[0] % 8]:
                                nc.vector.tensor_scalar(
                                    out=dst, in0=ps[:],
                                    scalar1=0.0, scalar2=0.0,
                                    op0=OP.max, op1=OP.add,
                                    accum_out=acc,
                                )
                            else:
                                nc.scalar.activation(
                                    out=dst, in_=ps[:], func=AF.Relu,
                                    accum_out=acc,
                                )
                            ei_oi[0] += 1
                    for j in range(4):
                        b = 4 * g + j
                        nc.vector.tensor_tensor(
                            out=gmat[b][:].rearrange("p (c k) -> p c k", c=5),
                            in0=gmask[:].rearrange("p (c k) -> p c k", c=5),
                            in1=usum[b][:, :, None].broadcast_to([128, 5, K]),
                            op=OP.mult,
                        )

                def conv_io(g):
                    for q in range(4):
                        for j in range(4):
                            b = 4 * g + j
                            ps = pio.tile([128, NO], F32, name="psio", tag="psio")
                            for (o0, o1) in ((0, 512), (512, NO)):
                                nc.tensor.matmul(
                                    ps[:, o0:o1],
                                    xT4[g][32 * j:32 * j + 17,
                                           128 * q:128 * (q + 1)],
                                    w4[32 * j:32 * j + 17, o0:o1],
                                    start=True, stop=True,
                                    tile_position=(32 * j, 0),
                                )
                            dst = yr_io[b][:, NO * q:NO * (q + 1)]
                            if IO_DVE_PAT[ei_io[0] % 8]:
                                nc.vector.tensor_scalar(
                                    out=dst, in0=ps[:],
                                    scalar1=0.0, scalar2=None, op0=OP.max,
                                )
                            else:
                                nc.scalar.activation(
                                    out=dst, in_=ps[:], func=AF.Relu,
                                )
                            ei_io[0] += 1

                ps_l = [None, None]

                def logits(g):
                    ps_l[g] = poi.tile([128, N], F32, name="ps_l", tag="ps")
                    for c in range(5):
                        for j in range(4):
                            b = 4 * g + j
                            nc.tensor.matmul(
                                ps_l[g][32 * j:32 * j + K, :],
                                gmat[b][:, K * c:K * (c + 1)],
                                yr_oi[b][:, N * c:N * (c + 1)],
                                start=(c == 0), stop=(c == 4),
                                tile_position=(0, 32 * j),
                            )

                def softmax(g):
                    nc.vector.tensor_reduce(
                        out=negmax[g][:], in_=ps_l[g][:],
                        op=OP.max, axis=mybir.AxisListType.X, negate=True,
                    )
                    nc.scalar.activation(
                        out=exp_sb[g][:], in_=ps_l[g][:], func=AF.Exp,
                        bias=negmax[g][:], scale=1.0,
                        accum_out=zsum[g][:],
                    )
                    nc.vector.reciprocal(rz[g][:], zsum[g][:])
                    nc.vector.scalar_tensor_tensor(
                        out=cb_sb[g][:], in0=exp_sb[g][:],
                        scalar=rz[g][:], in1=bbias[:],
                        op0=OP.mult, op1=OP.add,
                    )

                def cb_transpose(g):
                    tr = poi.tile([128, N], F32, name="tr", tag="ps")
                    for q in range(4):
                        nc.tensor.transpose(
                            tr[:, 128 * q:128 * (q + 1)],
                            cb_sb[g][:, 128 * q:128 * (q + 1)],
                            ident,
                        )
                    # extract cols 32a+k (k<10) of each q-chunk -> ebt[g][:, (q,a,k)]
                    src = tr[:].rearrange("m (q a kk) -> m q a kk", q=4, a=4)[
                        :, :, :, 0:K]
                    dst = ebt[g][:].rearrange("m (q a k) -> m q a k", q=4, a=4)
                    nc.vector.tensor_copy(dst, src)

                ps_s = [None, None]   # (slo, shi) per g

                def s_matmul(g):
                    slo = poi.tile([128, N], F32, name="slo", tag="ps")
                    shi = poi.tile([128, N], F32, name="shi", tag="ps")
                    ps_s[g] = (slo, shi)
                    for q in range(4):
                        for j in range(4):
                            b = 4 * g + j
                            eb = ebt[g][:, 40 * q + K * j:40 * q + K * (j + 1)]
                            nc.tensor.matmul(
                                slo[32 * j:32 * j + K, :],
                                eb,
                                yr_io[b][:, NO * q:NO * q + 512],
                                start=(q == 0), stop=(q == 3),
                                tile_position=(0, 32 * j),
                            )
                            nc.tensor.matmul(
                                shi[32 * j:32 * j + K, 0:128],
                                eb,
                                yr_io[b][:, NO * q + 512:NO * (q + 1)],
                                start=(q == 0), stop=(q == 3),
                                tile_position=(0, 32 * j),
                            )

                def s_tail(g):
                    slo, shi = ps_s[g]
                    nc.vector.tensor_tensor(
                        out=sm_s[g][:, 0:512], in0=slo[:],
                        in1=smask[:, 0:512], op=OP.mult,
                    )
                    nc.vector.tensor_tensor(
                        out=sm_s[g][:, 512:NO], in0=shi[:, 0:128],
                        in1=smask[:, 512:NO], op=OP.mult,
                    )
                    nc.scalar.activation(
                        out=sq_s[g][:], in_=sm_s[g][:], func=AF.Square,
                        accum_out=n2[:, g:g + 1],
                    )

                # ---- pipeline ----
                conv_oi(0)
                conv_oi(1)
                conv_io(0)
                logits(0)
                softmax(0)
                conv_io(1)
                logits(1)
                softmax(1)
                cb_transpose(0)
                s_matmul(0)
                cb_transpose(1)
                s_matmul(1)
                s_tail(0)
                s_tail(1)

                # out = n2/(n2+1)
                nc.vector.tensor_scalar(out=t_c[:], in0=n2[:],
                                        scalar1=1.0, scalar2=None, op0=OP.add)
                nc.vector.reciprocal(t_d[:], t_c[:])
                nc.vector.tensor_tensor(out=t_o[:], in0=n2[:], in1=t_d[:],
                                        op=OP.mult)
                for g in range(2):
                    for j in range(4):
                        eng = nc.sync if j % 2 == 0 else nc.gpsimd
                        eng.dma_start(
                            out_d[4 * g + j:4 * g + j + 1, :],
                            t_o[32 * j:32 * j + K, g:g + 1],
                        )
    nc.compile()
    return nc


_PROGRAM_CACHE = None
_CONSTS = None


def _get_program():
    global _PROGRAM_CACHE
    if _PROGRAM_CACHE is None:
        _PROGRAM_CACHE = _build_program()
    return _PROGRAM_CACHE


def kernel(timecaps, conv_w, conv_b, B_bias):
    global _CONSTS
    timecaps = np.ascontiguousarray(np.asarray(timecaps, dtype=np.float32))
    conv_w = np.ascontiguousarray(np.asarray(conv_w, dtype=np.float32))
    conv_b = np.ascontiguousarray(np.asarray(conv_b, dtype=np.float32))
    B_bias = np.ascontiguousarray(np.asarray(B_bias, dtype=np.float32))
    if _CONSTS is None:
        _CONSTS = _consts_np()

    nc = _get_program()
    in_maps = [
        {
            "x": timecaps[core * BPC:(core + 1) * BPC],
            "w": conv_w,
            "cb": conv_b,
            "bb": B_bias,
            "cst": _CONSTS,
        }
        for core in range(NCORES)
    ]
    res = run_bass_kernel_spmd(nc, in_maps, list(range(NCORES)))
    out = np.concatenate([res.results[i]["out"] for i in range(NCORES)], axis=0)
    return out.reshape(B_FULL, K, 1).astype(np.float32)


if __name__ == "__main__":
    rng = np.random.default_rng(0)
    ins = {
        "timecaps": rng.standard_normal((B_FULL, N, DT), dtype=np.float32),
        "conv_w": (rng.standard_normal((DT, 1, NO), dtype=np.float32) * 0.05),
        "conv_b": np.zeros((NO,), dtype=np.float32),
        "B_bias": (rng.standard_normal((K, 1, N), dtype=np.float32) * 0.05),
    }
    print(kernel(**ins)[:2, :, 0])


# revision 16
# speedup vs baseline: 1.0039x; 1.0039x over previous
"""Trainium2 Bass kernel for nn_Classifier (capsule-style conv + routing).

Math (validated against the jax reference):
  W = conv_w[:,0,:]                                   # [16, 640]
  y[b,i,o]   = relu(sum_t x[b,i,t] W[t,o] + conv_b[o])          (conv as matmul, K=16)
  U[b,k,i,d] = y[b,i,k*64+d]
  Usum[b,k,d]= sum_i U[b,k,i,d]
  logits     = (U . Usum)/4            -> stable softmax over i  -> C
  Cb         = C + B_bias[k,i]
  S[b,k,:]   = sum_i Cb[b,k,i] U[b,k,i,:]
  out[b,k]   = n2/(n2+1),  n2 = |S|^2     (sqrt factor n/(n+eps) ~ 1, err < 1e-5)

Sharding: data-parallel over batch, 8 batches per core, 8 cores (SPMD).

Per-core design (b = 4g+j, g in {0,1} bgroups, j in 0..3):
  - constants (identity / gmask / smask / ones) precomputed on host, DMA'd in
  - x loaded naturally, PE-transposed (17-col chunks incl. a ones column for
    the bias fold) into xT4[g] [128,512]: rows 32j+0=1.0, 32j+1+t = x[b,:,t]
  - conv computed in BOTH orientations on PE (fp32r, 4x row-tiled via
    tile_position): yr_oi[b] [o-chunk(128) x i(512)] and yr_io[b]
    [i-chunk(128) x o(640)]
  - PSUM->SBUF relu evictions split across DVE and ACT (the hard bottleneck:
    PSUM reads are 1 elem/lane/cycle on each); usum via accum_out on the oi
    evictions; gmat/Cb/squash-accum offloaded to GpSimd (SBUF-only engine)
  - logits = gmat^T @ yr_oi (col-tiled 4x, accumulated over 5 o-chunks),
    softmax via reduce_max(negate) + exp(bias=-max, accum=Z), Cb=C/Z+B
  - Cb transposed on PE, S = sum_q CbT_q^T @ yr_io_q (col-tiled 4x)
  - PSUM: one 4-bank ring shared by oi/logits/CbT/S tiles + one 2x2-bank ring
    for io tiles = exactly 8 banks, no pool-scope barriers mid-kernel
"""

import numpy as np

import concourse.bass as bass
import concourse.mybir as mybir
import concourse.tile as tile
from concourse import bacc
from concourse.bass_utils import run_bass_kernel_spmd

F32 = mybir.dt.float32
F32R = mybir.dt.float32r

B_FULL = 64
N = 512          # num timecaps (routing dim m/i)
DT = 16          # dim timecaps (conv contraction)
K = 10           # classes
D = 64           # dim classes
NO = K * D       # 640 conv output channels
NCORES = 8
BPC = B_FULL // NCORES   # 8 batches per core

# consts layout (columns of the [128, CW] host-precomputed block)
C_IDENT = 0      # [128] identity for PE transpose
C_ONES = 128     # [4] ones columns for the xn bias fold
C_GMASK = 132    # [50] gmask: 0.25 at class(128c+p)==k
C_SMASK = 182    # [640] rows 32j+k: 1.0 at cols [64k,64k+64)
CW = 822

# eviction engine split (True -> DVE, False -> ACT), tuned for balance
OI_DVE_PAT = (1, 0, 1, 0, 1, 0, 1, 1)   # 5/8 of oi tiles on DVE
IO_DVE_PAT = (1, 0, 0, 1, 0, 0, 0, 0)   # 2/8 of io tiles on DVE


def _consts_np() -> np.ndarray:
    c = np.zeros((128, CW), np.float32)
    c[:, C_IDENT:C_IDENT + 128] = np.eye(128, dtype=np.float32)
    c[:, C_ONES:C_ONES + 4] = 1.0
    for cc in range(5):
        for p in range(128):
            k = (128 * cc + p) // 64
            c[p, C_GMASK + 10 * cc + k] = 0.25
    for j in range(4):
        for k in range(K):
            c[32 * j + k, C_SMASK + 64 * k:C_SMASK + 64 * (k + 1)] = 1.0
    return c


def _build_program():
    nc = bacc.Bacc("TRN2", target_bir_lowering=False)
    x_in = nc.declare_dram_parameter("x", [BPC, N, DT], F32, isOutput=False)
    w_in = nc.declare_dram_parameter("w", [DT, 1, NO], F32, isOutput=False)
    cb_in = nc.declare_dram_parameter("cb", [NO], F32, isOutput=False)
    bb_in = nc.declare_dram_parameter("bb", [K, 1, N], F32, isOutput=False)
    cst_in = nc.declare_dram_parameter("cst", [128, CW], F32, isOutput=False)
    out_d = nc.declare_dram_parameter("out", [BPC, K], F32, isOutput=True)

    AF = mybir.ActivationFunctionType
    OP = mybir.AluOpType

    with tile.TileContext(nc) as tc:
        with tc.tile_pool(name="sb", bufs=1) as sb:
            # ---- SBUF persistent tiles ----
            csb = sb.tile([128, CW], F32, name="csb", tag="csb")
            w4 = sb.tile([128, NO], F32R, name="w4", tag="w4")
            wstage = sb.tile([128, NO], F32, name="wstage", tag="wstage")
            bbias = sb.tile([128, N], F32, name="bbias", tag="bbias")
            xc_all = sb.tile([128, BPC * 64], F32, name="xc_all", tag="xc_all")
            xT4 = [sb.tile([128, N], F32R, name=f"xT4_{g}", tag=f"xT4_{g}")
                   for g in range(2)]
            yr_oi = [sb.tile([128, 5 * N], F32, name=f"yoi{b}", tag=f"yoi{b}")
                     for b in range(BPC)]
            yr_io = [sb.tile([128, 4 * NO], F32, name=f"yio{b}", tag=f"yio{b}")
                     for b in range(BPC)]
            usum = [sb.tile([128, 5], F32, name=f"us{b}", tag=f"us{b}")
                    for b in range(BPC)]
            gmat = [sb.tile([128, 5 * K], F32, name=f"gm{b}", tag=f"gm{b}")
                    for b in range(BPC)]
            exp_sb = [sb.tile([128, N], F32, name=f"ex{g}", tag=f"ex{g}") for g in range(2)]
            cb_sb = [sb.tile([128, N], F32, name=f"cb{g}", tag=f"cb{g}") for g in range(2)]
            negmax = [sb.tile([128, 1], F32, name=f"nm{g}", tag=f"nm{g}") for g in range(2)]
            zsum = [sb.tile([128, 1], F32, name=f"z{g}", tag=f"z{g}") for g in range(2)]
            rz = [sb.tile([128, 1], F32, name=f"rz{g}", tag=f"rz{g}") for g in range(2)]
            ebt = [sb.tile([128, 4 * 4 * K], F32, name=f"ebt{g}", tag=f"ebt{g}")
                   for g in range(2)]
            sm_s = [sb.tile([128, NO], F32, name=f"sm{g}", tag=f"sm{g}") for g in range(2)]
            sq_s = [sb.tile([128, NO], F32, name=f"sq{g}", tag=f"sq{g}") for g in range(2)]
            n2 = sb.tile([128, 2], F32, name="n2", tag="n2")
            t_c = sb.tile([128, 2], F32, name="t_c", tag="t_c")
            t_d = sb.tile([128, 2], F32, name="t_d", tag="t_d")
            t_o = sb.tile([128, 2], F32, name="t_o", tag="t_o")
            wrm = sb.tile([128, N], F32R, name="wrm", tag="wrm")

            ident = csb[:, C_IDENT:C_IDENT + 128]
            gmask = csb[:, C_GMASK:C_GMASK + 50]
            smask = csb[:, C_SMASK:C_SMASK + NO]

            # ---- input DMAs ----
            nc.gpsimd.memset(wrm[:].bitcast(F32), 0.125)
            for g in range(2):
                nc.gpsimd.memset(xT4[g][:].bitcast(F32), 1.0)
            nc.sync.dma_start(csb[:, 0:128], cst_in[:, 0:128])      # ident first
            nc.sync.dma_start(csb[:, 128:CW], cst_in[:, 128:CW])
            wflat = w_in.rearrange("t u o -> (t u) o")
            cbflat = cb_in.rearrange("(u o) -> u o", u=1)
            for j in range(4):
                nc.sync.dma_start(wstage[32 * j:32 * j + 16, :], wflat[:, :])
                nc.sync.dma_start(wstage[32 * j + 16:32 * j + 17, :], cbflat[:, :])
                nc.sync.dma_start(
                    bbias[32 * j:32 * j + K, :],
                    bb_in.rearrange("k u m -> k (u m)"),
                )
            engs = [nc.gpsimd, nc.sync, nc.scalar]
            for b in range(BPC):
                engs[b % 3].dma_start(
                    xc_all[:, 64 * b:64 * (b + 1)],
                    x_in[b].rearrange("(p w) t -> p (w t)", p=128),
                )

            nc.vector.tensor_copy(w4[:], wstage[:])

            # ---- x transposes: xn [i-part, 17u-chunks] -> xT4[g] [32j+c, i] ----
            with tc.tile_pool(name="px", bufs=4, space="PSUM") as px:
                ps_w = px.tile([128, N], F32, name="ps_w", tag="warm", bufs=1)
                for _ in range(12):
                    nc.tensor.matmul(
                        ps_w[:], wrm[0:17, 0:128], wrm[0:17, :],
                        start=True, stop=True,
                    )
                for g in range(2):
                    for j in range(4):
                        b = 4 * g + j
                        ps_x = px.tile([16, N], F32, name="ps_x", tag="ps_x")
                        for il in range(4):
                            nc.tensor.transpose(
                                ps_x[:, 128 * il:128 * (il + 1)],
                                xc_all[:, 64 * b + 16 * il:64 * b + 16 * il + 16],
                                ident,
                            )
                        # ps_x cols 128*il+p  ->  xT4 col 4*p+il (i = 4p+il)
                        dst = xT4[g][32 * j:32 * j + 16, :].rearrange(
                            "t (p il) -> t il p", il=4)
                        srcv = ps_x[:].rearrange("t (il p) -> t il p", il=4)
                        if j % 2 == 0:
                            nc.vector.tensor_copy(dst, srcv)
                        else:
                            nc.scalar.copy(dst, srcv)

            # ---- main PSUM rings: 4x1-bank (oi/logits/CbT/S) + 2x2-bank (io) ----
            with tc.tile_pool(name="poi", bufs=4, space="PSUM") as poi, \
                 tc.tile_pool(name="pio", bufs=2, space="PSUM") as pio:

                ei_oi = [0]
                ei_io = [0]

                def conv_oi(g):
                    for c in range(5):
                        for j in range(4):
                            b = 4 * g + j
                            ps = poi.tile([128, N], F32, name="ps", tag="ps")
                            nc.tensor.matmul(
                                ps[:],
                                w4[32 * j:32 * j + 17,
                                   128 * c:128 * (c + 1)],
                                xT4[g][32 * j:32 * j + 17, :],
                                start=True, stop=True,
                                tile_position=(32 * j, 0),
                            )
                            dst = yr_oi[b][:, N * c:N * (c + 1)]
                            acc = usum[b][:, c:c + 1]
                            if OI_DVE_PAT[ei_oi[0] % 8]:
                                nc.vector.tensor_scalar(
                                    out=dst, in0=ps[:],
                                    scalar1=0.0, scalar2=0.0,
                                    op0=OP.max, op1=OP.add,
                                    accum_out=acc,
                                )
                            else:
                                nc.scalar.activation(
                                    out=dst, in_=ps[:], func=AF.Relu,
                                    accum_out=acc,
                                )
                            ei_oi[0] += 1
                    for j in range(4):
                        b = 4 * g + j
                        nc.vector.tensor_tensor(
                            out=gmat[b][:].rearrange("p (c k) -> p c k", c=5),
                            in0=gmask[:].rearrange("p (c k) -> p c k", c=5),
                            in1=usum[b][:, :, None].broadcast_to([128, 5, K]),
                            op=OP.mult,
                        )

                def conv_io(g):
                    for q in range(4):
                        for j in range(4):
                            b = 4 * g + j
                            ps = pio.tile([128, NO], F32, name="psio", tag="psio")
                            for (o0, o1) in ((0, 512), (512, NO)):
                                nc.tensor.matmul(
                                    ps[:, o0:o1],
                                    xT4[g][32 * j:32 * j + 17,
                                           128 * q:128 * (q + 1)],
                                    w4[32 * j:32 * j + 17, o0:o1],
                                    start=True, stop=True,
                                    tile_position=(32 * j, 0),
                                )
                            dst = yr_io[b][:, NO * q:NO * (q + 1)]
                            if IO_DVE_PAT[ei_io[0] % 8]:
                                nc.vector.tensor_scalar(
                                    out=dst, in0=ps[:],
                                    scalar1=0.0, scalar2=None, op0=OP.max,
                                )
                            else:
                                nc.scalar.activation(
                                    out=dst, in_=ps[:], func=AF.Relu,
                                )
                            ei_io[0] += 1

                ps_l = [None, None]

                def logits(g):
                    ps_l[g] = poi.tile([128, N], F32, name="ps_l", tag="ps")
                    for c in range(5):
                        for j in range(4):
                            b = 4 * g + j
                            nc.tensor.matmul(
                                ps_l[g][32 * j:32 * j + K, :],
                                gmat[b][:, K * c:K * (c + 1)],
                                yr_oi[b][:, N * c:N * (c + 1)],
                                start=(c == 0), stop=(c == 4),
                                tile_position=(0, 32 * j),
                            )

                def softmax(g):
                    nc.vector.tensor_reduce(
                        out=negmax[g][:], in_=ps_l[g][:],
                        op=OP.max, axis=mybir.AxisListType.X, negate=True,
                    )
                    nc.scalar.activation(
                        out=exp_sb[g][:], in_=ps_l[g][:], func=AF.Exp,
                        bias=negmax[g][:], scale=1.0,
                        accum_out=zsum[g][:],
                    )
                    nc.vector.reciprocal(rz[g][:], zsum[g][:])
                    nc.vector.scalar_tensor_tensor(
                        out=cb_sb[g][:], in0=exp_sb[g][:],
                        scalar=rz[g][:], in1=bbias[:],
                        op0=OP.mult, op1=OP.add,
                    )

                def cb_transpose(g):
                    tr = poi.tile([128, N], F32, name="tr", tag="ps")
                    for q in range(4):
                        nc.tensor.transpose(
                            tr[:, 128 * q:128 * (q + 1)],
                            cb_sb[g][:, 128 * q:128 * (q + 1)],
                            ident,
                        )
                    # extract cols 32a+k (k<10) of each q-chunk -> ebt[g][:, (q,a,k)]
                    src = tr[:].rearrange("m (q a kk) -> m q a kk", q=4, a=4)[
                        :, :, :, 0:K]
                    dst = ebt[g][:].rearrange("m (q a k) -> m q a k", q=4, a=4)
                    nc.vector.tensor_copy(dst, src)

                ps_s = [None, None]   # (slo, shi) per g

                def s_matmul(g):
                    slo = poi.tile([128, N], F32, name="slo", tag="ps")
                    shi = poi.tile([128, N], F32, name="shi", tag="ps")
                    ps_s[g] = (slo, shi)
                    for q in range(4):
                        for j in range(4):
                            b = 4 * g + j
                            eb = ebt[g][:, 40 * q + K * j:40 * q + K * (j + 1)]
                            nc.tensor.matmul(
                                slo[32 * j:32 * j + K, :],
                                eb,
                                yr_io[b][:, NO * q:NO * q + 512],
                                start=(q == 0), stop=(q == 3),
                                tile_position=(0, 32 * j),
                            )
                            nc.tensor.matmul(
                                shi[32 * j:32 * j + K, 0:128],
                                eb,
                                yr_io[b][:, NO * q + 512:NO * (q + 1)],
                                start=(q == 0), stop=(q == 3),
                                tile_position=(0, 32 * j),
                            )

                def s_tail(g):
                    slo, shi = ps_s[g]
                    nc.vector.tensor_tensor(
                        out=sm_s[g][:, 0:512], in0=slo[:],
                        in1=smask[:, 0:512], op=OP.mult,
                    )
                    nc.vector.tensor_tensor(
                        out=sm_s[g][:, 512:NO], in0=shi[:, 0:128],
                        in1=smask[:, 512:NO], op=OP.mult,
                    )
                    nc.scalar.activation(
                        out=sq_s[g][:], in_=sm_s[g][:], func=AF.Square,
                        accum_out=n2[:, g:g + 1],
                    )

                # ---- pipeline ----
                conv_oi(0)
                conv_oi(1)
                conv_io(0)
                logits(0)
                softmax(0)
                conv_io(1)
                logits(1)
                softmax(1)
                cb_transpose(0)
                s_matmul(0)
                cb_transpose(1)
                s_matmul(1)
                s_tail(0)
                s_tail(1)

                # out = n2/(n2+1)
                nc.vector.tensor_scalar(out=t_c[:], in0=n2[:],
                                        scalar1=1.0, scalar2=None, op0=OP.add)
                nc.vector.reciprocal(t_d[:], t_c[:])
                nc.vector.tensor_tensor(out=t_o[:], in0=n2[:], in1=t_d[:],
                                        op=OP.mult)
                for g in range(2):
                    for j in range(4):
                        eng = nc.sync if j % 2 == 0 else nc.gpsimd
                        eng.dma_start(
                            out_d[4 * g + j:4 * g + j + 1, :],
                            t_o[32 * j:32 * j + K, g:g + 1],
                        )
    nc.compile()
    return nc


_PROGRAM_CACHE = None
_CONSTS = None


def _get_program():
    global _PROGRAM_CACHE
    if _PROGRAM_CACHE is None:
        _PROGRAM_CACHE = _build_program()
    return _PROGRAM_CACHE


def kernel(timecaps, conv_w, conv_b, B_bias):
    global _CONSTS
    timecaps = np.ascontiguousarray(np.asarray(timecaps, dtype=np.float32))
    conv_w = np.ascontiguousarray(np.asarray(conv_w, dtype=np.float32))
    conv_b = np.ascontiguousarray(np.asarray(conv_b, dtype=np.float32))
    B_bias = np.ascontiguousarray(np.asarray(B_bias, dtype=np.float32))
    if _CONSTS is None:
        _CONSTS = _consts_np()

    nc = _get_program()
    in_maps = [
        {
            "x": timecaps[core * BPC:(core + 1) * BPC],
            "w": conv_w,
            "cb": conv_b,
            "bb": B_bias,
            "cst": _CONSTS,
        }
        for core in range(NCORES)
    ]
    res = run_bass_kernel_spmd(nc, in_maps, list(range(NCORES)))
    out = np.concatenate([res.results[i]["out"] for i in range(NCORES)], axis=0)
    return out.reshape(B_FULL, K, 1).astype(np.float32)


if __name__ == "__main__":
    rng = np.random.default_rng(0)
    ins = {
        "timecaps": rng.standard_normal((B_FULL, N, DT), dtype=np.float32),
        "conv_w": (rng.standard_normal((DT, 1, NO), dtype=np.float32) * 0.05),
        "conv_b": np.zeros((NO,), dtype=np.float32),
        "B_bias": (rng.standard_normal((K, 1, N), dtype=np.float32) * 0.05),
    }
    print(kernel(**ins)[:2, :, 0])
